# revision 6
# baseline (speedup 1.0000x reference)
"""Allegro GNN layer on 8 Trainium2 NeuronCores.

Strategy: group edges by 128-node sender chunk on host (sanctioned by the
sharding hint) so each core owns a contiguous 1024-node range and the
segment_sum is fully local.  Scatter/gather are expressed as matmuls against
one-hot selector matrices fed as data, so the device graph is static and
identical across cores (SPMD).  Matmul operands are bf16 (full PE rate);
accumulation is fp32 in PSUM.
"""

import math
import os
import sys

import numpy as np

sys.path.insert(0, "/opt/trn_rl_repo")

NUM_NODES = 8192
MUL = 64
ENV_P = 6
EPS = 1.0 / math.sqrt(17.0)
N_CORES = 8
CHUNK_NODES = 128
CHUNKS_PER_CORE = 8

ENV_A = -(ENV_P + 1) * (ENV_P + 2) / 2.0  # -28
ENV_B = float(ENV_P * (ENV_P + 2))  # 48
ENV_C = -ENV_P * (ENV_P + 1) / 2.0  # -21

_GRAPH_CACHE = {}
_FLAGS_PATCHED = [False]


def _patch_cc_flags():
    # neuronx-cc's DataLocalityOpt pass crashes on some DMA patterns
    # (assert NeuronLocalTensor in splitAndRetile); skip it.
    if _FLAGS_PATCHED[0]:
        return
    try:
        from concourse.compiler_utils import (get_compiler_flags,
                                              set_compiler_flags)
        flags = get_compiler_flags()
        out = []
        for f in flags:
            if f.startswith("--tensorizer-options=") and "DataLocalityOpt" not in f:
                f = f.rstrip() + " --skip-pass=DataLocalityOpt "
            out.append(f)
        set_compiler_flags(out)
    except Exception:
        pass
    _FLAGS_PATCHED[0] = True


def _build_graph(T):
    """Build (and cache) the Bass graph for tiles-per-chunk T."""
    if T in _GRAPH_CACHE:
        return _GRAPH_CACHE[T]
    _patch_cc_flags()

    import concourse.tile as tile
    from concourse import bacc, mybir
    from contextlib import ExitStack

    f32 = mybir.dt.float32
    bf16 = mybir.dt.bfloat16
    NT = CHUNKS_PER_CORE * T
    E_PAD = NT * 128
    SPC = T * 128
    groups = []
    e0 = 0
    while e0 < SPC:
        eg = min(512, SPC - e0)
        groups.append((e0, eg))
        e0 += eg

    nc = bacc.Bacc("TRN2", target_bir_lowering=False, debug=False,
                   num_devices=N_CORES)

    xT = nc.dram_tensor("xT", [512, E_PAD], bf16, kind="ExternalInput").ap()
    vec = nc.dram_tensor("vec", [E_PAD, 3], f32, kind="ExternalInput").ap()
    VTm = nc.dram_tensor("VTm", [64, 4, E_PAD], bf16, kind="ExternalInput").ap()
    S = nc.dram_tensor("S", [NT, 128, 128], bf16, kind="ExternalInput").ap()
    STg = nc.dram_tensor("STg", [CHUNKS_PER_CORE, 128, SPC], bf16,
                         kind="ExternalInput").ap()
    W1p = nc.dram_tensor("W1p", [128, 5, 512], bf16, kind="ExternalInput").ap()
    W2p = nc.dram_tensor("W2p", [128, 4, 512], bf16, kind="ExternalInput").ap()
    W3p = nc.dram_tensor("W3p", [128, 4, 512], bf16, kind="ExternalInput").ap()
    Wwp = nc.dram_tensor("Wwp", [128, 4, 64], bf16, kind="ExternalInput").ap()
    WLp = nc.dram_tensor("WLp", [64, 3, 64], bf16, kind="ExternalInput").ap()
    out1 = nc.dram_tensor("out1", [E_PAD, 512], f32, kind="ExternalOutput").ap()
    out2 = nc.dram_tensor("out2", [64, 3, E_PAD], f32, kind="ExternalOutput").ap()

    MM = mybir.AluOpType.mult
    AD = mybir.AluOpType.add

    with tile.TileContext(nc) as tc:
        with ExitStack() as stack:
            consts = stack.enter_context(tc.tile_pool(name="consts", bufs=1))
            w1s = consts.tile([128, 5, 512], bf16)
            nc.sync.dma_start(w1s[:], W1p[:])
            w2s = consts.tile([128, 4, 512], bf16)
            nc.sync.dma_start(w2s[:], W2p[:])
            w3s = consts.tile([128, 4, 512], bf16)
            nc.sync.dma_start(w3s[:], W3p[:])
            wls = consts.tile([64, 3, 64], bf16)
            nc.sync.dma_start(wls[:], WLp[:])

            res = stack.enter_context(tc.tile_pool(name="res", bufs=1))
            w_all = res.tile([128, NT * 64], bf16)
            y_all = res.tile([128, NT, 4], bf16)
            env_all = res.tile([128, NT], f32)

            # ---- Phase A: per-edge w, Y, envelope -----------------------
            with ExitStack() as st_a:
                pA = st_a.enter_context(tc.tile_pool(name="pA", bufs=3))
                pAc = st_a.enter_context(tc.tile_pool(name="pAc", bufs=1))
                pAw = st_a.enter_context(
                    tc.tile_pool(name="pAw", bufs=4, space="PSUM"))
                wws = pAc.tile([128, 4, 64], bf16)
                nc.sync.dma_start(wws[:], Wwp[:])
                for s in range(NT // 4):
                    xa = pA.tile([128, 4, 4, 128], bf16)
                    for kb in range(4):
                        nc.sync.dma_start(
                            xa[:, kb],
                            xT[kb * 128:(kb + 1) * 128,
                               s * 512:(s + 1) * 512].rearrange(
                                   "p (g e) -> p g e", g=4))
                    vc4 = pA.tile([128, 4, 3], f32)
                    nc.sync.dma_start(
                        vc4[:],
                        vec[s * 512:(s + 1) * 512, :].rearrange(
                            "(g p) d -> p g d", p=128))
                    for es in range(4):
                        pw = pAw.tile([128, 64], f32)
                        for kb in range(4):
                            nc.tensor.matmul(
                                pw[:], xa[:, kb, es], wws[:, kb],
                                start=(kb == 0), stop=(kb == 3))
                        nc.vector.tensor_copy(
                            w_all[:, (4 * s + es) * 64:(4 * s + es + 1) * 64],
                            pw[:])
                    sq = pA.tile([128, 4, 3], f32)
                    nc.vector.tensor_tensor(sq[:], vc4[:], vc4[:], op=MM)
                    r2 = pA.tile([128, 4], f32)
                    nc.vector.tensor_reduce(
                        r2[:], sq[:], axis=mybir.AxisListType.X, op=AD)
                    r = pA.tile([128, 4], f32)
                    nc.scalar.sqrt(r[:], r2[:])
                    rinv = pA.tile([128, 4], f32)
                    nc.vector.reciprocal(rinv[:], r[:])
                    rinv3 = pA.tile([128, 4], f32)
                    nc.scalar.mul(rinv3[:], rinv[:], math.sqrt(3.0))
                    nc.vector.memset(y_all[:, 4 * s:4 * s + 4, 0], 1.0)
                    nc.vector.tensor_tensor(
                        y_all[:, 4 * s:4 * s + 4, 1:4], vc4[:],
                        rinv3[:].rearrange("p (g o) -> p g o", o=1)
                        .to_broadcast([128, 4, 3]), op=MM)
                    inner = pA.tile([128, 4], f32)
                    nc.vector.tensor_scalar(
                        inner[:], r[:], scalar1=ENV_B, scalar2=ENV_A,
                        op0=MM, op1=AD)
                    nc.vector.scalar_tensor_tensor(
                        inner[:], in0=r2[:], scalar=ENV_C, in1=inner[:],
                        op0=MM, op1=AD)
                    u3 = pA.tile([128, 4], f32)
                    nc.vector.tensor_tensor(u3[:], r2[:], r[:], op=MM)
                    u6 = pA.tile([128, 4], f32)
                    nc.vector.tensor_tensor(u6[:], u3[:], u3[:], op=MM)
                    poly = pA.tile([128, 4], f32)
                    nc.vector.tensor_tensor(poly[:], u6[:], inner[:], op=MM)
                    nc.vector.tensor_scalar(
                        poly[:], poly[:], scalar1=1.0, scalar2=None, op0=AD)
                    mask = pA.tile([128, 4], f32)
                    nc.vector.tensor_scalar(
                        mask[:], r2[:], scalar1=1.0,
                        scalar2=1.0 / math.sqrt(512.0),
                        op0=mybir.AluOpType.is_lt, op1=MM)
                    nc.vector.tensor_tensor(
                        env_all[:, 4 * s:4 * s + 4], poly[:], mask[:], op=MM)

            # ---- Phase B/C/D per chunk ----------------------------------
            with ExitStack() as st_b:
                pStg = st_b.enter_context(tc.tile_pool(name="pStg", bufs=2))
                pS = st_b.enter_context(tc.tile_pool(name="pS", bufs=3))
                pWy = st_b.enter_context(tc.tile_pool(name="pWy", bufs=3))
                pAgg = st_b.enter_context(tc.tile_pool(name="pAgg", bufs=2))
                pAggP = st_b.enter_context(
                    tc.tile_pool(name="pAggP", bufs=2, space="PSUM"))
                pGP = st_b.enter_context(
                    tc.tile_pool(name="pGP", bufs=3, space="PSUM"))
                pG = st_b.enter_context(tc.tile_pool(name="pG", bufs=2))
                pV = st_b.enter_context(tc.tile_pool(name="pV", bufs=2))
                pX = st_b.enter_context(tc.tile_pool(name="pX", bufs=2))
                pCG = st_b.enter_context(tc.tile_pool(name="pCG", bufs=2))
                pVo = st_b.enter_context(tc.tile_pool(name="pVo", bufs=2))
                pH1 = st_b.enter_context(tc.tile_pool(name="pH1", bufs=2))
                pH2 = st_b.enter_context(tc.tile_pool(name="pH2", bufs=2))
                pMP = st_b.enter_context(
                    tc.tile_pool(name="pMP", bufs=2, space="PSUM"))
                pXo = st_b.enter_context(tc.tile_pool(name="pXo", bufs=2))
                for c in range(CHUNKS_PER_CORE):
                    stg = pStg.tile([128, SPC], bf16)
                    nc.sync.dma_start(stg[:], STg[c])
                    pagg = pAggP.tile([128, 256], f32)
                    for lt in range(T):
                        t = c * T + lt
                        st_ = pS.tile([128, 128], bf16)
                        nc.sync.dma_start(st_[:], S[t])
                        wy = pWy.tile([128, 4, 64], bf16)
                        nc.vector.tensor_tensor(
                            wy[:],
                            y_all[:, t, :].rearrange("p (k o) -> p k o", o=1)
                            .to_broadcast([128, 4, 64]),
                            w_all[:, t * 64:(t + 1) * 64]
                            .rearrange("p (o m) -> p o m", o=1)
                            .to_broadcast([128, 4, 64]),
                            op=MM)
                        nc.tensor.matmul(
                            pagg[:], st_[:],
                            wy[:].rearrange("p a b -> p (a b)"),
                            start=(lt == 0), stop=(lt == T - 1))
                    aggc = pAgg.tile([128, 256], bf16)
                    nc.vector.tensor_copy(aggc[:], pagg[:])

                    for (e0_, eg) in groups:
                        gsl = slice(c * SPC + e0_, c * SPC + e0_ + eg)
                        wyg = pG.tile([64, 4, 512], bf16)
                        for k in range(4):
                            pg = pGP.tile([64, 512], f32, tag="gv")
                            nc.tensor.matmul(
                                pg[:, :eg], aggc[:, k * 64:(k + 1) * 64],
                                stg[:, e0_:e0_ + eg], start=True, stop=True)
                            nc.vector.tensor_copy(wyg[:, k, :eg], pg[:, :eg])
                        vt = pV.tile([64, 4, 512], bf16)
                        nc.sync.dma_start(vt[:, :, :eg], VTm[:, :, gsl])
                        a0 = wyg[:, 0, :eg]
                        ax, ay, az = (wyg[:, 1, :eg], wyg[:, 2, :eg],
                                      wyg[:, 3, :eg])
                        b0 = vt[:, 0, :eg]
                        bx, by, bz = (vt[:, 1, :eg], vt[:, 2, :eg],
                                      vt[:, 3, :eg])
                        xtd = pX.tile([128, 5, 512], bf16)
                        for kb in range(4):
                            nc.sync.dma_start(
                                xtd[:, kb, :eg],
                                xT[kb * 128:(kb + 1) * 128, gsl])
                        nc.vector.tensor_tensor(xtd[0:64, 4, :eg], a0, b0, op=MM)
                        s2t = pCG.tile([64, 512], bf16)
                        tmp0 = pCG.tile([64, 512], bf16, tag="t0")
                        nc.vector.tensor_tensor(s2t[:, :eg], ax, bx, op=MM)
                        nc.vector.tensor_tensor(tmp0[:, :eg], ay, by, op=MM)
                        nc.vector.tensor_tensor(
                            s2t[:, :eg], s2t[:, :eg], tmp0[:, :eg], op=AD)
                        nc.vector.tensor_tensor(tmp0[:, :eg], az, bz, op=MM)
                        nc.vector.tensor_tensor(
                            s2t[:, :eg], s2t[:, :eg], tmp0[:, :eg], op=AD)
                        nc.sync.dma_start(xtd[64:128, 4, :eg], s2t[:, :eg])
                        # vecs + V_out
                        vout = pVo.tile([64, 3, 512], f32)
                        comps = [(bx, ax, (ay, bz, az, by)),
                                 (by, ay, (az, bx, ax, bz)),
                                 (bz, az, (ax, by, ay, bx))]
                        for ci, (b1c, a1c, (cp, cq, cr_, cs_)) in enumerate(comps):
                            t0 = pCG.tile([64, 512], bf16, tag="t0")
                            t1 = pCG.tile([64, 512], bf16, tag="t1")
                            t2 = pCG.tile([64, 512], bf16, tag="t2")
                            nc.vector.tensor_tensor(t2[:, :eg], cp, cq, op=MM)
                            nc.vector.tensor_tensor(t1[:, :eg], cr_, cs_, op=MM)
                            nc.vector.tensor_tensor(
                                t2[:, :eg], t2[:, :eg], t1[:, :eg],
                                op=mybir.AluOpType.subtract)
                            nc.vector.tensor_tensor(t0[:, :eg], a0, b1c, op=MM)
                            nc.vector.tensor_tensor(t1[:, :eg], a1c, b0, op=MM)
                            pv = pGP.tile([64, 512], f32, tag="gv")
                            nc.tensor.matmul(pv[:, :eg], wls[:, 0], t0[:, :eg],
                                             start=True, stop=False)
                            nc.tensor.matmul(pv[:, :eg], wls[:, 1], t1[:, :eg],
                                             start=False, stop=False)
                            nc.tensor.matmul(pv[:, :eg], wls[:, 2], t2[:, :eg],
                                             start=False, stop=True)
                            nc.vector.tensor_copy(vout[:, ci, :eg], pv[:, :eg])
                        nc.sync.dma_start(out2[:, :, gsl], vout[:, :, :eg])
                        # MLP
                        h1 = pH1.tile([128, 4, 512], bf16)
                        for hb in range(4):
                            p1 = pMP.tile([128, 512], f32, tag="mlp")
                            for kb in range(5):
                                nc.tensor.matmul(
                                    p1[:, :eg],
                                    w1s[:, kb, hb * 128:(hb + 1) * 128],
                                    xtd[:, kb, :eg],
                                    start=(kb == 0), stop=(kb == 4))
                            nc.scalar.activation(
                                h1[:, hb, :eg], p1[:, :eg],
                                mybir.ActivationFunctionType.Silu,
                                scale=1.0 / math.sqrt(640.0))
                        h2 = pH2.tile([128, 4, 512], bf16)
                        for hb in range(4):
                            p2 = pMP.tile([128, 512], f32, tag="mlp")
                            for kb in range(4):
                                nc.tensor.matmul(
                                    p2[:, :eg],
                                    w2s[:, kb, hb * 128:(hb + 1) * 128],
                                    h1[:, kb, :eg],
                                    start=(kb == 0), stop=(kb == 3))
                            nc.scalar.activation(
                                h2[:, hb, :eg], p2[:, :eg],
                                mybir.ActivationFunctionType.Silu,
                                scale=1.0 / math.sqrt(512.0))
                        for es in range(eg // 128):
                            gt = (c * SPC + e0_) // 128 + es
                            p3 = pMP.tile([128, 512], f32, tag="mlp")
                            for kb in range(4):
                                nc.tensor.matmul(
                                    p3[:],
                                    h2[:, kb, es * 128:(es + 1) * 128],
                                    w3s[:, kb],
                                    start=(kb == 0), stop=(kb == 3))
                            xo = pXo.tile([128, 512], f32)
                            nc.vector.tensor_scalar_mul(
                                xo[:], p3[:], env_all[:, gt:gt + 1])
                            nc.sync.dma_start(
                                out1[gt * 128:(gt + 1) * 128, :], xo[:])
    nc.compile()
    _GRAPH_CACHE[T] = (nc, NT, E_PAD, SPC)
    return _GRAPH_CACHE[T]


def _prep_host(vectors, x, V, senders, W_w, W1, W2, W3, W_lin):
    import ml_dtypes
    bf = ml_dtypes.bfloat16
    E = senders.shape[0]
    gchunk = senders.astype(np.int64) // CHUNK_NODES  # 0..63
    order = np.argsort(gchunk, kind="stable")
    counts = np.bincount(gchunk, minlength=64)
    T = max(18, int(math.ceil(counts.max() / 128.0)))
    SPC = T * 128
    NT = CHUNKS_PER_CORE * T
    E_PAD = NT * 128

    sg = gchunk[order]
    starts = np.zeros(64, np.int64)
    starts[1:] = np.cumsum(counts)[:-1]
    within = np.arange(E, dtype=np.int64) - starts[sg]
    core = sg // CHUNKS_PER_CORE
    lchunk = sg % CHUNKS_PER_CORE
    slot = lchunk * SPC + within

    W1s = W1.astype(np.float64)
    W1s[576:640] /= math.sqrt(3.0)
    W1p = np.ascontiguousarray(
        W1s.reshape(5, 128, 512).transpose(1, 0, 2)).astype(bf)
    W2p = np.ascontiguousarray(
        W2.reshape(4, 128, 512).transpose(1, 0, 2)).astype(bf)
    W3p = np.ascontiguousarray(
        W3.reshape(4, 128, 512).transpose(1, 0, 2)).astype(bf)
    Wwp = np.ascontiguousarray(
        (W_w.astype(np.float64) * (EPS / math.sqrt(512.0)))
        .reshape(4, 128, 64).transpose(1, 0, 2)).astype(bf)
    WLs = W_lin.astype(np.float64) / math.sqrt(192.0)
    WLs[128:192] /= math.sqrt(2.0)
    WLp = np.ascontiguousarray(
        WLs.reshape(3, 64, 64).transpose(1, 0, 2)).astype(bf)

    in_maps = []
    metas = []
    for d in range(N_CORES):
        m = core == d
        eidx = order[m]
        sl = slot[m]
        fill = eidx[0] if len(eidx) else 0
        xs = np.empty((E_PAD, 512), np.float32)
        xs[:] = x[fill]
        xs[sl] = x[eidx]
        vs = np.empty((E_PAD, 3), np.float32)
        vs[:] = vectors[fill]
        vs[sl] = vectors[eidx]
        Vs = np.empty((E_PAD, 64, 4), np.float32)
        Vs[:] = V[fill]
        Vs[sl] = V[eidx]
        xTc = np.ascontiguousarray(xs.T).astype(bf)
        VTmc = np.ascontiguousarray(Vs.transpose(1, 2, 0)).astype(bf)
        Sc = np.zeros((NT, 128, 128), bf)
        col = (senders[eidx] % CHUNK_NODES).astype(np.int64)
        Sc[sl // 128, sl % 128, col] = 1.0
        STgc = np.zeros((CHUNKS_PER_CORE, 128, SPC), bf)
        STgc[sl // SPC, col, sl % SPC] = 1.0
        in_maps.append({
            "xT": xTc, "vec": vs, "VTm": VTmc, "S": Sc, "STg": STgc,
            "W1p": W1p, "W2p": W2p, "W3p": W3p, "Wwp": Wwp, "WLp": WLp,
        })
        metas.append((eidx, sl))
    return T, in_maps, metas


def kernel(vectors, x, V, senders, W_w, W1, W2, W3, W_lin):
    vectors = np.asarray(vectors, np.float32)
    x = np.asarray(x, np.float32)
    V = np.asarray(V, np.float32)
    senders = np.asarray(senders)
    W_w = np.asarray(W_w, np.float32)
    W1 = np.asarray(W1, np.float32)
    W2 = np.asarray(W2, np.float32)
    W3 = np.asarray(W3, np.float32)
    W_lin = np.asarray(W_lin, np.float32)

    T, in_maps, metas = _prep_host(
        vectors, x, V, senders, W_w, W1, W2, W3, W_lin)
    nc, NT, E_PAD, SPC = _build_graph(T)

    from concourse.bass_utils import run_bass_kernel_spmd
    res = run_bass_kernel_spmd(nc, in_maps, core_ids=list(range(N_CORES)))

    E = senders.shape[0]
    x_out = np.empty((E, 512), np.float32)
    V_out = np.empty((E, 64, 3), np.float32)
    for d in range(N_CORES):
        eidx, sl = metas[d]
        o1 = res.results[d]["out1"]
        o2 = res.results[d]["out2"]  # [64, 3, E_PAD]
        x_out[eidx] = o1[sl]
        V_out[eidx] = o2.transpose(2, 0, 1)[sl]
    return x_out, V_out
